# revision 13
# baseline (speedup 1.0000x reference)
"""Trainium2 Bass kernel for LatentGNN-style ChannelAttention.

Reference computation (per batch element b, on full inputs):
  v    = 8x8 block-mean pool of x[b]            [C=512, S=256]
  A1_k = softmax_c((v @ psi_k).T)               [D=100, C]    (k = 0, 1)
  z_k  = A1_k @ v                               [D, S]
  zn   = z / (||z||_2 + 1e-6)   (rows of stacked z [200, S])
  G    = softmax_n(zn @ zn.T)                   [200, 200]
  zp   = G @ z                                  [200, S]
  A2_k = softmax_d(v @ phi_k)                   [C, D]
  out  = sum_k A2_k @ zp_k                      [C, S]
  attn = sigmoid(mean_s(v + out @ out_w))       [C]

Folds used here (exact up to fp reassociation):
  - mean_s(out @ out_w) == out @ wbar,  wbar = mean_t(out_w)  -> no [C,S]@[S,S]
  - softmax normalization deferred through the following matmul (scale rows
    after, or fold into the tiny q vector).

Sharding: pure data parallel, one batch element per NeuronCore (8 cores).
"""

import numpy as np

import concourse.bacc as bacc
import concourse.bass as bass
import concourse.mybir as mybir
import concourse.tile as tile
from concourse.bass import ts
from concourse.bass_utils import run_bass_kernel_spmd

F32 = mybir.dt.float32
AF = mybir.ActivationFunctionType
AX = mybir.AxisListType

B, C, H, W = 8, 512, 128, 128
S = 256          # pooled spatial size (16*16)
D = 100          # latent dim per kernel
K = 2            # num kernels
P = 128          # partitions
NQ = C // P      # 4 channel chunks
HW = H * W       # 16384
TW = 4096        # x-tile free size: 32 h-rows
NT = HW // TW    # 4 tiles per channel chunk


def build_bass(repeat=1):
    nc = bacc.Bacc(trn_type="TRN2", target_bir_lowering=False, debug=False)

    xl = nc.dram_tensor("xl", [C, HW], F32, kind="ExternalInput").ap()
    psi = nc.dram_tensor("psi", [P, K * 2 * D], F32, kind="ExternalInput").ap()
    phi = nc.dram_tensor("phi", [P, K * 2 * D], F32, kind="ExternalInput").ap()
    wbar = nc.dram_tensor("wbar", [P, S], F32, kind="ExternalInput").ap()
    eye = nc.dram_tensor("eye", [P, P], F32, kind="ExternalInput").ap()
    attn = nc.dram_tensor("attn", [NQ, P], F32, kind="ExternalOutput").ap()

    with tile.TileContext(nc) as tc, (
        tc.tile_pool(name="xpool", bufs=6)) as xpool, (
        tc.tile_pool(name="singles", bufs=1)) as singles, (
        tc.tile_pool(name="sp", bufs=2)) as sp, (
        tc.tile_pool(name="ps_big", bufs=2, space="PSUM")) as ps_big, (
        tc.tile_pool(name="ps_m1", bufs=2, space="PSUM")) as ps_m1, (
        tc.tile_pool(name="ps_tr", bufs=2, space="PSUM")) as ps_tr, (
        tc.tile_pool(name="ps_tiny", bufs=1, space="PSUM")) as ps_tiny:

        # ---- persistent small tensors ----
        ident = singles.tile([P, P], F32)
        nc.sync.dma_start(out=ident, in_=eye)

        psi_s = singles.tile([P, K, 2, D], F32)     # [p, k, s-chunk, d]
        nc.sync.dma_start(out=psi_s, in_=psi.rearrange("p (k r d) -> p k r d", k=K, r=2))
        phi_s = singles.tile([P, K, 2, D], F32)
        nc.sync.dma_start(out=phi_s, in_=phi.rearrange("p (k r d) -> p k r d", k=K, r=2))
        wbarB = singles.tile([P, S], F32)           # wbar broadcast to all partitions
        nc.sync.dma_start(out=wbarB, in_=wbar)

        V = singles.tile([P, NQ, S], F32)           # v, channel chunk q on [:, q, :]
        VT = singles.tile([P, 2, C], F32)           # v.T, s-chunk r on [:, r, :]
        vbar = singles.tile([P, NQ], F32)           # raw row sums of v
        ET = singles.tile([P, K, NQ, D], F32)       # exp(M1).T chunks
        A2T = singles.tile([D, K, NQ, P], F32)      # A2.T chunks
        Z = singles.tile([D, K, S], F32)            # z_k rows
        ZN = singles.tile([D, K, S], F32)           # normalized z rows
        ZT = singles.tile([P, 2, K * D], F32)       # zn.T, s-chunk r
        EGT = singles.tile([D, 2, K * D], F32)      # exp(G).T, n-chunk nh
        QK = singles.tile([D, K], F32)              # q vectors
        RG = singles.tile([D, K], F32)              # 1/rowsum(exp(G))
        ATT = singles.tile([P, NQ], F32)

        for _rep in range(repeat):
            _kernel_body(nc, tc, xpool, singles, sp, ps_big, ps_m1, ps_tr, ps_tiny,
                         ident, psi_s, phi_s, wbarB, V, VT, vbar, ET, A2T, Z, ZN,
                         ZT, EGT, QK, RG, ATT, xl, attn)

    nc.compile()
    return nc


def _kernel_body(nc, tc, xpool, singles, sp, ps_big, ps_m1, ps_tr, ps_tiny,
                 ident, psi_s, phi_s, wbarB, V, VT, vbar, ET, A2T, Z, ZN,
                 ZT, EGT, QK, RG, ATT, xl, attn):
    if True:
        m1 = [ps_m1.tile([D, C], F32, tag="m1", name=f"m1_{k}") for k in range(K)]

        # ---- phase A: pooling (DMA-bound) + per-chunk matmuls ----
        for q in range(NQ):
            for t in range(NT):
                xt = xpool.tile([P, TW], F32, tag="xt", name=f"xt_{q}_{t}")
                nc.sync.dma_start(out=xt, in_=xl[ts(q, P), ts(t, TW)])
                xv = xt.rearrange("p (i di j dj) -> p i j di dj", i=4, di=8, j=16, dj=8)
                # sum over the 8x8 block (di, dj innermost) -> [p, i, j]
                nc.vector.reduce_sum(out=V[:, q, ts(t, 64)], in_=xv, axis=AX.XY)
            nc.vector.tensor_scalar_mul(out=V[:, q, :], in0=V[:, q, :], scalar1=1.0 / 64.0)
            nc.vector.reduce_sum(out=vbar[:, q:q + 1], in_=V[:, q, :], axis=AX.X)
            # vT chunk
            for r in range(2):
                tp = ps_tr.tile([P, P], F32, tag="tr", name=f"vt_ps_{q}_{r}")
                nc.tensor.transpose(tp, V[:, q, ts(r, P)], ident)
                nc.scalar.copy(out=VT[:, r, ts(q, P)], in_=tp)
            # M1 column block: m1[k][:, q*128:...] = psi_k.T @ vT cols
            for k in range(K):
                for r in range(2):
                    nc.tensor.matmul(
                        m1[k][:, ts(q, P)], lhsT=psi_s[:, k, r, :], rhs=VT[:, r, ts(q, P)],
                        start=(r == 0), stop=(r == 1))
            # A2 block for this channel chunk: softmax_d(v @ phi_k)
            for k in range(K):
                p2 = ps_big.tile([P, D], F32, tag="big", name=f"p2_{q}_{k}")
                for r in range(2):
                    nc.tensor.matmul(
                        p2, lhsT=VT[:, r, ts(q, P)], rhs=phi_s[:, k, r, :],
                        start=(r == 0), stop=(r == 1))
                nmax2 = sp.tile([P, 1], F32, tag="nmax2", name=f"nmax2_{q}_{k}")
                nc.vector.reduce_max(out=nmax2, in_=p2, axis=AX.X, negate=True)
                e2 = sp.tile([P, D], F32, tag="e2", name=f"e2_{q}_{k}")
                s2 = sp.tile([P, 1], F32, tag="s2", name=f"s2_{q}_{k}")
                nc.scalar.activation(out=e2, in_=p2, func=AF.Exp, bias=nmax2, accum_out=s2)
                rs2 = sp.tile([P, 1], F32, tag="rs2", name=f"rs2_{q}_{k}")
                nc.vector.reciprocal(out=rs2, in_=s2)
                a2 = sp.tile([P, D], F32, tag="a2", name=f"a2_{q}_{k}")
                nc.vector.tensor_scalar_mul(out=a2, in0=e2, scalar1=rs2)
                tp2 = ps_tr.tile([D, P], F32, tag="tr", name=f"a2t_ps_{q}_{k}")
                nc.tensor.transpose(tp2, a2, ident)
                nc.scalar.copy(out=A2T[:, k, q, :], in_=tp2)

        # ---- phase B: latent GNN on [D,*] tiles ----
        for k in range(K):
            # softmax over c of m1[k]; defer row normalization into z
            nmax = sp.tile([D, 1], F32, tag="nmax", name=f"nmax_{k}")
            nc.vector.reduce_max(out=nmax, in_=m1[k], axis=AX.X, negate=True)
            e1 = sp.tile([D, C], F32, tag="e1", name=f"e1_{k}")
            sm = sp.tile([D, 1], F32, tag="sm", name=f"sm_{k}")
            nc.scalar.activation(out=e1, in_=m1[k], func=AF.Exp, bias=nmax, accum_out=sm)
            rs = sp.tile([D, 1], F32, tag="rs", name=f"rs_{k}")
            nc.vector.reciprocal(out=rs, in_=sm)
            for q in range(NQ):
                tp = ps_tr.tile([P, D], F32, tag="tr", name=f"et_ps_{k}_{q}")
                nc.tensor.transpose(tp, e1[:, ts(q, P)], ident[:D, :D])
                nc.scalar.copy(out=ET[:, k, q, :], in_=tp)
            zps = ps_big.tile([D, S], F32, tag="big", name=f"zps_{k}")
            for q in range(NQ):
                nc.tensor.matmul(zps, lhsT=ET[:, k, q, :], rhs=V[:, q, :],
                                 start=(q == 0), stop=(q == NQ - 1))
            nc.vector.tensor_scalar_mul(out=Z[:, k, :], in0=zps, scalar1=rs)
            # zn = z / (||z|| + 1e-6)
            sq = sp.tile([D, S], F32, tag="sq", name=f"sq_{k}")
            nc.vector.tensor_mul(out=sq, in0=Z[:, k, :], in1=Z[:, k, :])
            n2 = sp.tile([D, 1], F32, tag="n2", name=f"n2_{k}")
            nc.vector.reduce_sum(out=n2, in_=sq, axis=AX.X)
            nrm = sp.tile([D, 1], F32, tag="nrm", name=f"nrm_{k}")
            nc.scalar.sqrt(out=nrm, in_=n2)
            nc.vector.tensor_scalar_add(out=nrm, in0=nrm, scalar1=1e-6)
            rn = sp.tile([D, 1], F32, tag="rn", name=f"rn_{k}")
            nc.vector.reciprocal(out=rn, in_=nrm)
            nc.vector.tensor_scalar_mul(out=ZN[:, k, :], in0=Z[:, k, :], scalar1=rn)
            for r in range(2):
                tp = ps_tr.tile([P, D], F32, tag="tr", name=f"znt_ps_{k}_{r}")
                nc.tensor.transpose(tp, ZN[:, k, ts(r, P)], ident[:D, :D])
                nc.scalar.copy(out=ZT[:, r, ts(k, D)], in_=tp)

        # G = softmax_n(zn @ zn.T), row half mh at a time
        for mh in range(K):
            g = ps_big.tile([D, K * D], F32, tag="big", name=f"g_{mh}")
            for r in range(2):
                nc.tensor.matmul(g, lhsT=ZT[:, r, ts(mh, D)], rhs=ZT[:, r, :],
                                 start=(r == 0), stop=(r == 1))
            nmaxg = sp.tile([D, 1], F32, tag="nmaxg", name=f"nmaxg_{mh}")
            nc.vector.reduce_max(out=nmaxg, in_=g, axis=AX.X, negate=True)
            eg = sp.tile([D, K * D], F32, tag="eg", name=f"eg_{mh}")
            sg = sp.tile([D, 1], F32, tag="sg", name=f"sg_{mh}")
            nc.scalar.activation(out=eg, in_=g, func=AF.Exp, bias=nmaxg, accum_out=sg)
            nc.vector.reciprocal(out=RG[:, mh:mh + 1], in_=sg)
            for nh in range(K):
                tp = ps_tr.tile([D, D], F32, tag="tr", name=f"egt_ps_{mh}_{nh}")
                nc.tensor.transpose(tp, eg[:, ts(nh, D)], ident[:D, :D])
                nc.scalar.copy(out=EGT[:, nh, ts(mh, D)], in_=tp)

        # zp_raw_k = exp(G)_k-rows @ z ; q_k = rg * (zp_raw_k @ wbar)
        for k in range(K):
            zp = ps_big.tile([D, S], F32, tag="big", name=f"zp_{k}")
            for nh in range(K):
                nc.tensor.matmul(zp, lhsT=EGT[:, nh, ts(k, D)], rhs=Z[:, nh, :],
                                 start=(nh == 0), stop=(nh == 1))
            prod = sp.tile([D, S], F32, tag="prod", name=f"prod_{k}")
            nc.vector.tensor_mul(out=prod, in0=zp, in1=wbarB[:D, :])
            qr = sp.tile([D, 1], F32, tag="qr", name=f"qr_{k}")
            nc.vector.reduce_sum(out=qr, in_=prod, axis=AX.X)
            nc.vector.tensor_mul(out=QK[:, k:k + 1], in0=qr, in1=RG[:, k:k + 1])

        # obar_q = sum_k A2T_kq.T @ q_k ; attn = sigmoid(vbar/S + obar)
        for q in range(NQ):
            ob = ps_tiny.tile([P, 1], F32, tag="tiny", name=f"ob_{q}")
            for k in range(K):
                nc.tensor.matmul(ob, lhsT=A2T[:, k, q, :], rhs=QK[:, k:k + 1],
                                 start=(k == 0), stop=(k == K - 1))
            obs = sp.tile([P, 1], F32, tag="obs", name=f"obs_{q}")
            nc.scalar.copy(out=obs, in_=ob)
            nc.scalar.activation(out=ATT[:, q:q + 1], in_=vbar[:, q:q + 1],
                                 func=AF.Sigmoid, bias=obs, scale=1.0 / S)

        # transpose ATT [128, 4] -> [4, 128] so the DRAM write is contiguous
        att_ps = ps_tiny.tile([NQ, P], F32, tag="tiny2", name="att_ps")
        nc.tensor.transpose(att_ps, ATT, ident)
        att_sb = sp.tile([NQ, P], F32, tag="att_sb", name="att_sb")
        nc.scalar.copy(out=att_sb, in_=att_ps)
        nc.sync.dma_start(out=attn, in_=att_sb)


_CACHE = {}


def _get_nc():
    if "nc" not in _CACHE:
        _CACHE["nc"] = build_bass()
    return _CACHE["nc"]


def kernel(x, psi_w, phi_w, out_w):
    x = np.ascontiguousarray(np.asarray(x, dtype=np.float32))
    psi_w = np.asarray(psi_w, dtype=np.float32)
    phi_w = np.asarray(phi_w, dtype=np.float32)
    out_w = np.asarray(out_w, dtype=np.float32)

    # host-side packing of the (replicated, tiny) parameters
    # psi_w [K, 256, D] -> [128, K, s-chunk, D]
    psiP = np.ascontiguousarray(
        psi_w.reshape(K, 2, P, D).transpose(2, 0, 1, 3).reshape(P, K * 2 * D))
    phiP = np.ascontiguousarray(
        phi_w.reshape(K, 2, P, D).transpose(2, 0, 1, 3).reshape(P, K * 2 * D))
    wbar = out_w.mean(axis=1).astype(np.float32)          # [256]
    wbarB = np.ascontiguousarray(np.broadcast_to(wbar[None, :], (P, S)))

    eye = np.eye(P, dtype=np.float32)
    nc = _get_nc()
    in_maps = [
        {"xl": x[b].reshape(C, HW), "psi": psiP, "phi": phiP, "wbar": wbarB,
         "eye": eye}
        for b in range(B)
    ]
    res = run_bass_kernel_spmd(nc, in_maps, core_ids=list(range(B)))
    out = np.stack([r["attn"].reshape(C) for r in res.results])
    return out.reshape(B, C, 1, 1).astype(np.float32)
